# revision 22
# baseline (speedup 1.0000x reference)
"""SSD DecodeLayer (box decode + per-anchor class max/argmax) on 8 trn2 cores.

Data-parallel over batch: each of the 8 cores processes 16 of the 128 batches.

Layout per core: the 16 batches are processed in NTILES=4 tiles of BPT=4
batches. A tile's 4*8732 = 34928 anchors are contiguous in DRAM (batch-major),
and are split as 118 partitions x 296 anchors of *flat* (batch, anchor) index:
partition p holds flat anchors [p*296, (p+1)*296) of the tile -> every DMA is
one contiguous chunk per partition (29.6KB in, 1-5KB out). The per-anchor
constant tables are gathered host-side with n = (p*296 + a) % 8732, which is
tile-invariant since 34928 % 8732 == 0.

Per-anchor math (ch = 4 loc + 21 classes):
  scores  = max_j cls_j                      (f32 reduce over class axis)
  eq_j    = cls_j >= scores                  (bf16 0/1)
  r       = max_j eq_j * (21-j)              (bf16; first-occurrence argmax)
  classes = 21 - r, valid = sign(21 - r), num = per-batch sum(valid)
  boxes   = clip(xy -+ bwh/2) with xy = loc01*wh + cxcy,
            bwh/2 = exp(loc23) * wh/2
"""

import numpy as np
import ml_dtypes

B, N, NCH = 128, 8732, 25
NCLS = 21          # 20 classes + background
NCORES = 8
BPC = B // NCORES  # batches per core
P = 118
BPT = 4            # batches per tile
NTILES = BPC // BPT
AT = BPT * N // P  # 296 flat anchors per partition per tile
CP = NCLS + 1      # padded class width (even, for bf16 2x mode)

ALL_BLOCKS = ("scores", "argmax", "decode", "num")


HALF = AT // 2  # batch boundaries align to half-partitions: N = 29.5 * AT


def batch_of_half(p, s):
    """Tile-batch index owning half-row (partition p, half s)."""
    return (2 * p + s) * HALF // N


_CACHE = {}


def build_program(repeat=1, blocks=ALL_BLOCKS):
    import concourse.bass as bass
    import concourse.bacc as bacc
    import concourse.mybir as mybir
    import concourse.tile as tile

    blocks = set(blocks)
    f32 = mybir.dt.float32
    bf16 = mybir.dt.bfloat16
    i32 = mybir.dt.int32
    u8 = mybir.dt.uint8
    Alu = mybir.AluOpType
    Act = mybir.ActivationFunctionType
    X = mybir.AxisListType.X

    nc = bacc.Bacc("TRN2", target_bir_lowering=False, debug=False)

    lg = nc.dram_tensor("logits", [BPC, N, NCH], f32, kind="ExternalInput")
    w1 = nc.dram_tensor("w1", [P, AT * 2], f32, kind="ExternalInput")
    c1 = nc.dram_tensor("c1", [P, AT * 2], f32, kind="ExternalInput")
    l1 = nc.dram_tensor("l1", [P, AT * 2], f32, kind="ExternalInput")
    wq = nc.dram_tensor("wq", [P, AT * CP], bf16, kind="ExternalInput")
    mk = nc.dram_tensor("msk", [P, 2 * BPT], f32, kind="ExternalInput")

    boxes_o = nc.dram_tensor("boxes", [BPC, N, 4], f32, kind="ExternalOutput")
    scores_o = nc.dram_tensor("scores", [BPC, N], f32, kind="ExternalOutput")
    classes_o = nc.dram_tensor("classes", [BPC, N], i32, kind="ExternalOutput")
    valid_o = nc.dram_tensor("valid", [BPC, N], u8, kind="ExternalOutput")
    num_o = nc.dram_tensor("num", [BPC], i32, kind="ExternalOutput")

    # Flat views: tile t, partition p holds flat anchors t*P*AT + p*AT + [0,AT)
    lgv = (
        lg.ap().rearrange("b n c -> (b n) c")
        .rearrange("(t p a) c -> p t (a c)", t=NTILES, p=P)
    )  # [P, NTILES, AT*25]
    bv = (
        boxes_o.ap().rearrange("b n c -> (b n) c")
        .rearrange("(t p a) c -> p t (a c)", t=NTILES, p=P)
    )  # [P, NTILES, AT*4]
    sv = scores_o.ap().rearrange("b n -> (b n)").rearrange(
        "(t p a) -> p t a", t=NTILES, p=P
    )  # [P, NTILES, AT]
    cv = classes_o.ap().rearrange("b n -> (b n)").rearrange(
        "(t p a) -> p t a", t=NTILES, p=P
    )
    vv = valid_o.ap().rearrange("b n -> (b n)").rearrange(
        "(t p a) -> p t a", t=NTILES, p=P
    )
    # num[b] with b = t*BPT + bb, laid out [bb, t] for the PSUM result DMA
    nvt = num_o.ap().rearrange("(t bb) -> bb t", bb=BPT)  # [4, 4]

    with tile.TileContext(nc) as tc:
        with (
            tc.tile_pool(name="consts", bufs=1) as consts,
            tc.tile_pool(name="lp", bufs=2) as lp,
            tc.tile_pool(name="eqp", bufs=2) as eqp,
            tc.tile_pool(name="wp", bufs=2) as wp,
            tc.tile_pool(name="pp", bufs=1, space="PSUM") as pp,
        ):
            w1t = consts.tile([P, AT, 2], f32, tag="w1t")
            c1t = consts.tile([P, AT, 2], f32, tag="c1t")
            l1t = consts.tile([P, AT, 2], f32, tag="l1t")
            wqt = consts.tile([P, AT * CP], bf16, tag="wqt")
            nc.sync.dma_start(out=w1t[:].rearrange("p a c -> p (a c)"), in_=w1.ap())
            nc.sync.dma_start(out=c1t[:].rearrange("p a c -> p (a c)"), in_=c1.ap())
            nc.sync.dma_start(out=l1t[:].rearrange("p a c -> p (a c)"), in_=l1.ap())
            nc.sync.dma_start(out=wqt[:], in_=wq.ap())

            # one accum column per (half s, tile t): [P, 2, NTILES]
            numpart = consts.tile([P, 2, NTILES], f32, tag="numpart")
            nc.vector.memset(numpart[:], 0.0)
            b21 = consts.tile([P, 1], f32, tag="b21")
            nc.vector.memset(b21[:], float(NCLS))
            mkt = consts.tile([P, 2 * BPT], f32, tag="mkt")
            nc.sync.dma_start(out=mkt[:], in_=mk.ap())

            # dummy store sources for disabled blocks (perf experiments only)
            if not {"scores", "argmax", "decode"} <= blocks:
                df32 = consts.tile([P, AT * 4], f32, tag="df32")
                di32 = consts.tile([P, AT], i32, tag="di32")
                du8 = consts.tile([P, AT], u8, tag="du8")
                nc.vector.memset(df32[:], 0.0)
                nc.vector.memset(di32[:], 0)
                nc.vector.memset(du8[:], 0)

            for _rep in range(repeat):
                for t in range(NTILES):
                    lt = lp.tile([P, AT * NCH], f32, tag="lt")
                    nc.sync.dma_start(out=lt[:], in_=lgv[:, t, :])
                    l3 = lt[:].rearrange("p (x c) -> p x c", c=NCH)  # [P,AT,25]
                    cls3 = l3[:, :, 4:NCH]                            # [P,AT,21]
                    loc01 = l3[:, :, 0:2]
                    loc23 = l3[:, :, 2:4]

                    # ---- scores = max over classes (f32, exact) ----
                    if "scores" in blocks:
                        m = wp.tile([P, AT], f32, tag="m")
                        nc.vector.reduce_max(out=m[:], in_=cls3, axis=X)
                        nc.sync.dma_start(out=sv[:, t, :], in_=m[:])

                    # ---- argmax via eq * weight, first occurrence wins ----
                    if "argmax" in blocks:
                        eq = eqp.tile([P, AT, CP], bf16, tag="eq")
                        nc.gpsimd.memset(eq[:, :, NCLS:CP], 0.0)
                        m_ap = m[:]
                        mb = bass.AP(
                            tensor=m_ap.tensor,
                            offset=m_ap.offset,
                            ap=list(m_ap.ap) + [[0, NCLS]],
                        )  # [P, AT, 21] broadcast over class axis
                        nc.vector.tensor_tensor(
                            out=eq[:, :, 0:NCLS], in0=cls3, in1=mb, op=Alu.is_ge,
                        )
                        eqf = eq[:].rearrange("p a c -> p (a c)")
                        nc.vector.tensor_mul(eqf, eqf, wqt[:])
                        r = wp.tile([P, AT], bf16, tag="r")
                        nc.vector.reduce_max(out=r[:], in_=eq[:], axis=X)

                        ci = wp.tile([P, AT], i32, tag="ci")
                        nc.scalar.activation(
                            out=ci[:], in_=r[:], func=Act.Identity, bias=b21[:],
                            scale=-1.0,
                        )
                        nc.sync.dma_start(out=cv[:, t, :], in_=ci[:])
                        # valid; per-half-row counts (batch = f(partition, half))
                        vld = wp.tile([P, AT], u8, tag="vld")
                        for s in range(2):
                            nc.scalar.activation(
                                out=vld[:, s * HALF : (s + 1) * HALF],
                                in_=r[:, s * HALF : (s + 1) * HALF],
                                func=Act.Sign, bias=b21[:], scale=-1.0,
                                accum_out=numpart[:, s, t : t + 1],
                            )
                        nc.sync.dma_start(out=vv[:, t, :], in_=vld[:])
                    else:  # keep identical DMA traffic
                        nc.sync.dma_start(out=cv[:, t, :], in_=di32[:])
                        nc.sync.dma_start(out=vv[:, t, :], in_=du8[:])
                    if "scores" not in blocks:
                        nc.sync.dma_start(out=sv[:, t, :], in_=df32[:, 0:AT])

                    # ---- box decode ----
                    if "decode" in blocks:
                        xy = wp.tile([P, AT, 2], f32, tag="xy")
                        nc.gpsimd.tensor_mul(xy[:], loc01, w1t[:])
                        nc.gpsimd.tensor_add(xy[:], xy[:], c1t[:])
                        bw = wp.tile([P, AT, 2], f32, tag="bw")
                        nc.scalar.activation(
                            out=bw[:].rearrange("p a c -> p (a c)"),
                            in_=loc23, func=Act.Exp,
                        )
                        nc.gpsimd.tensor_mul(bw[:], bw[:], l1t[:])
                        bx = wp.tile([P, AT, 4], f32, tag="bx")
                        nc.gpsimd.tensor_sub(bx[:, :, 0:2], xy[:], bw[:])
                        nc.gpsimd.tensor_add(bx[:, :, 2:4], xy[:], bw[:])
                        bxf = bx[:].rearrange("p a c -> p (a c)")
                        nc.vector.tensor_scalar(
                            out=bxf, in0=bxf, scalar1=0.0, scalar2=1.0,
                            op0=Alu.max, op1=Alu.min,
                        )
                        nc.sync.dma_start(out=bv[:, t, :], in_=bxf)
                    else:
                        nc.sync.dma_start(out=bv[:, t, :], in_=df32[:])

            # ---- num[bb, t] = sum_p sum_s msk[p, s*4+bb] * numpart[p, s, t]
            if "num" in blocks and "argmax" in blocks:
                ps = pp.tile([BPT, NTILES], f32, tag="ps")
                nc.tensor.matmul(
                    ps[:], mkt[:, 0:BPT], numpart[:, 0, :],
                    start=True, stop=False,
                )
                nc.tensor.matmul(
                    ps[:], mkt[:, BPT : 2 * BPT], numpart[:, 1, :],
                    start=False, stop=True,
                )
                numi = consts.tile([BPT, NTILES], i32, tag="numi")
                nc.scalar.copy(out=numi[:], in_=ps[:])
                nc.sync.dma_start(out=nvt, in_=numi[:])
            else:
                numi = consts.tile([BPT, NTILES], i32, tag="numi")
                nc.vector.memset(numi[:], 0)
                nc.sync.dma_start(out=nvt, in_=numi[:])

    nc.compile()
    return nc


def host_tables(anchors):
    a = np.asarray(anchors, np.float32)
    cxcy = (a[:, 2:4] + a[:, 0:2]) * 0.5
    wh = a[:, 2:4] - a[:, 0:2]
    wh2 = wh * 0.5  # bwh/2 = exp(loc23) * wh/2

    # anchor id for (partition, slot): tile-invariant since P*AT % N == 0
    idx = (np.arange(P)[:, None] * AT + np.arange(AT)[None, :]) % N  # [P, AT]

    def lay(t):  # [N,2] -> [P, AT*2]
        return np.ascontiguousarray(t[idx]).reshape(P, AT * 2)

    wvals = np.array([NCLS - j for j in range(NCLS)] + [0], np.float32)
    wq = np.tile(wvals[None, None], (P, AT, 1)).reshape(P, AT * CP)

    msk = np.zeros((P, 2 * BPT), np.float32)
    for p in range(P):
        for s in range(2):
            msk[p, s * BPT + batch_of_half(p, s)] = 1.0

    return {
        "w1": lay(wh),
        "c1": lay(cxcy),
        "l1": lay(wh2),
        "wq": wq.astype(ml_dtypes.bfloat16),
        "msk": msk,
    }


LAST_RESULT = None


def kernel(logits, anchors):
    global LAST_RESULT
    from concourse.bass_utils import run_bass_kernel_spmd

    if "nc" not in _CACHE:
        _CACHE["nc"] = build_program()
    nc = _CACHE["nc"]

    logits = np.ascontiguousarray(np.asarray(logits, np.float32))
    tables = host_tables(anchors)
    shards = logits.reshape(NCORES, BPC, N, NCH)
    in_maps = [
        {"logits": np.ascontiguousarray(shards[i]), **tables}
        for i in range(NCORES)
    ]
    res = run_bass_kernel_spmd(nc, in_maps, core_ids=list(range(NCORES)))
    LAST_RESULT = res

    boxes = np.concatenate([r["boxes"] for r in res.results], axis=0)
    scores = np.concatenate([r["scores"] for r in res.results], axis=0)
    classes = np.concatenate([r["classes"] for r in res.results], axis=0)
    valid = np.concatenate([r["valid"] for r in res.results], axis=0)
    num = np.concatenate([r["num"] for r in res.results], axis=0)
    return boxes, scores, classes.astype(np.int32), valid.astype(bool), num.astype(np.int32)


# revision 28
# speedup vs baseline: 1.9129x; 1.9129x over previous
"""SSD DecodeLayer (box decode + per-anchor class max/argmax) on 8 trn2 cores.

Data-parallel over batch: each of the 8 cores processes 16 of the 128 batches.

Layout per core: the 16 batches are processed in NTILES=4 tiles of BPT=4
batches. A tile's 4*8732 = 34928 anchors are contiguous in DRAM (batch-major),
and are split as 118 partitions x 296 anchors of *flat* (batch, anchor) index:
partition p holds flat anchors [p*296, (p+1)*296) of the tile -> every DMA is
one contiguous chunk per partition (29.6KB in, 1-5KB out). The per-anchor
constant tables are gathered host-side with n = (p*296 + a) % 8732, which is
tile-invariant since 34928 % 8732 == 0.

Per-anchor math (ch = 4 loc + 21 classes):
  scores  = max_j cls_j                      (f32 reduce over class axis)
  eq_j    = cls_j >= scores                  (bf16 0/1)
  r       = max_j eq_j * (21-j)              (bf16; first-occurrence argmax)
  classes = 21 - r, valid = sign(21 - r), num = per-batch sum(valid)
  boxes   = clip(xy -+ bwh/2) with xy = loc01*wh + cxcy,
            bwh/2 = exp(loc23) * wh/2
"""

import numpy as np
import ml_dtypes

B, N, NCH = 128, 8732, 25
NCLS = 21          # 20 classes + background
NCORES = 8
BPC = B // NCORES  # batches per core
P = 118
BPT = 4            # batches per tile
NTILES = BPC // BPT
AT = BPT * N // P  # 296 flat anchors per partition per tile
CP = NCLS + 1      # padded class width (even, for bf16 2x mode)

ALL_BLOCKS = ("scores", "argmax", "decode", "num")


HALF = AT // 2  # batch boundaries align to half-partitions: N = 29.5 * AT


def batch_of_half(p, s):
    """Tile-batch index owning half-row (partition p, half s)."""
    return (2 * p + s) * HALF // N


_CACHE = {}


# split_dma=True routes output stores through the ACT HWDGE ring so compute-
# gated store triggers don't queue ahead of the next tile's load trigger on the
# SP sequencer; load_bufs=3 deepens input prefetch. Both A/B'd on hardware
# (all five outputs verified exact, timing tied-to-slightly-better vs all-on-SP;
# the kernel sits at the shared-HBM bandwidth wall either way).
def build_program(repeat=1, blocks=ALL_BLOCKS, split_dma=True, load_bufs=3):
    import concourse.bass as bass
    import concourse.bacc as bacc
    import concourse.mybir as mybir
    import concourse.tile as tile

    blocks = set(blocks)
    f32 = mybir.dt.float32
    bf16 = mybir.dt.bfloat16
    i32 = mybir.dt.int32
    u8 = mybir.dt.uint8
    Alu = mybir.AluOpType
    Act = mybir.ActivationFunctionType
    X = mybir.AxisListType.X

    nc = bacc.Bacc("TRN2", target_bir_lowering=False, debug=False)

    lg = nc.dram_tensor("logits", [BPC, N, NCH], f32, kind="ExternalInput")
    w1 = nc.dram_tensor("w1", [P, AT * 2], f32, kind="ExternalInput")
    c1 = nc.dram_tensor("c1", [P, AT * 2], f32, kind="ExternalInput")
    l1 = nc.dram_tensor("l1", [P, AT * 2], f32, kind="ExternalInput")
    wq = nc.dram_tensor("wq", [P, AT * CP], bf16, kind="ExternalInput")
    mk = nc.dram_tensor("msk", [P, 2 * BPT], f32, kind="ExternalInput")

    boxes_o = nc.dram_tensor("boxes", [BPC, N, 4], f32, kind="ExternalOutput")
    scores_o = nc.dram_tensor("scores", [BPC, N], f32, kind="ExternalOutput")
    classes_o = nc.dram_tensor("classes", [BPC, N], i32, kind="ExternalOutput")
    valid_o = nc.dram_tensor("valid", [BPC, N], u8, kind="ExternalOutput")
    num_o = nc.dram_tensor("num", [BPC], i32, kind="ExternalOutput")

    # Flat views: tile t, partition p holds flat anchors t*P*AT + p*AT + [0,AT)
    lgv = (
        lg.ap().rearrange("b n c -> (b n) c")
        .rearrange("(t p a) c -> p t (a c)", t=NTILES, p=P)
    )  # [P, NTILES, AT*25]
    bv = (
        boxes_o.ap().rearrange("b n c -> (b n) c")
        .rearrange("(t p a) c -> p t (a c)", t=NTILES, p=P)
    )  # [P, NTILES, AT*4]
    sv = scores_o.ap().rearrange("b n -> (b n)").rearrange(
        "(t p a) -> p t a", t=NTILES, p=P
    )  # [P, NTILES, AT]
    cv = classes_o.ap().rearrange("b n -> (b n)").rearrange(
        "(t p a) -> p t a", t=NTILES, p=P
    )
    vv = valid_o.ap().rearrange("b n -> (b n)").rearrange(
        "(t p a) -> p t a", t=NTILES, p=P
    )
    # num[b] with b = t*BPT + bb, laid out [bb, t] for the PSUM result DMA
    nvt = num_o.ap().rearrange("(t bb) -> bb t", bb=BPT)  # [4, 4]

    with tile.TileContext(nc) as tc:
        # loads on the SP HWDGE ring, stores on the ACT HWDGE ring so that
        # compute-gated store triggers never queue ahead of the next tile's
        # load trigger on the same sequencer
        stq = nc.scalar if split_dma else nc.sync
        with (
            tc.tile_pool(name="consts", bufs=1) as consts,
            tc.tile_pool(name="lp", bufs=load_bufs) as lp,
            tc.tile_pool(name="eqp", bufs=2) as eqp,
            tc.tile_pool(name="wp", bufs=2) as wp,
            tc.tile_pool(name="pp", bufs=1, space="PSUM") as pp,
        ):
            w1t = consts.tile([P, AT, 2], f32, tag="w1t")
            c1t = consts.tile([P, AT, 2], f32, tag="c1t")
            l1t = consts.tile([P, AT, 2], f32, tag="l1t")
            wqt = consts.tile([P, AT * CP], bf16, tag="wqt")
            nc.sync.dma_start(out=w1t[:].rearrange("p a c -> p (a c)"), in_=w1.ap())
            nc.sync.dma_start(out=c1t[:].rearrange("p a c -> p (a c)"), in_=c1.ap())
            nc.sync.dma_start(out=l1t[:].rearrange("p a c -> p (a c)"), in_=l1.ap())
            nc.sync.dma_start(out=wqt[:], in_=wq.ap())

            # one accum column per (half s, tile t): [P, 2, NTILES]
            numpart = consts.tile([P, 2, NTILES], f32, tag="numpart")
            nc.vector.memset(numpart[:], 0.0)
            b21 = consts.tile([P, 1], f32, tag="b21")
            nc.vector.memset(b21[:], float(NCLS))
            mkt = consts.tile([P, 2 * BPT], f32, tag="mkt")
            nc.sync.dma_start(out=mkt[:], in_=mk.ap())

            # dummy store sources for disabled blocks (perf experiments only)
            if not {"scores", "argmax", "decode"} <= blocks:
                df32 = consts.tile([P, AT * 4], f32, tag="df32")
                di32 = consts.tile([P, AT], i32, tag="di32")
                du8 = consts.tile([P, AT], u8, tag="du8")
                nc.vector.memset(df32[:], 0.0)
                nc.vector.memset(di32[:], 0)
                nc.vector.memset(du8[:], 0)

            for _rep in range(repeat):
                for t in range(NTILES):
                    lt = lp.tile([P, AT * NCH], f32, tag="lt")
                    nc.sync.dma_start(out=lt[:], in_=lgv[:, t, :])
                    l3 = lt[:].rearrange("p (x c) -> p x c", c=NCH)  # [P,AT,25]
                    cls3 = l3[:, :, 4:NCH]                            # [P,AT,21]
                    loc01 = l3[:, :, 0:2]
                    loc23 = l3[:, :, 2:4]

                    # ---- scores = max over classes (f32, exact) ----
                    if "scores" in blocks:
                        m = wp.tile([P, AT], f32, tag="m")
                        nc.vector.reduce_max(out=m[:], in_=cls3, axis=X)
                        stq.dma_start(out=sv[:, t, :], in_=m[:])

                    # ---- argmax via eq * weight, first occurrence wins ----
                    if "argmax" in blocks:
                        eq = eqp.tile([P, AT, CP], bf16, tag="eq")
                        nc.gpsimd.memset(eq[:, :, NCLS:CP], 0.0)
                        m_ap = m[:]
                        mb = bass.AP(
                            tensor=m_ap.tensor,
                            offset=m_ap.offset,
                            ap=list(m_ap.ap) + [[0, NCLS]],
                        )  # [P, AT, 21] broadcast over class axis
                        nc.vector.tensor_tensor(
                            out=eq[:, :, 0:NCLS], in0=cls3, in1=mb, op=Alu.is_ge,
                        )
                        eqf = eq[:].rearrange("p a c -> p (a c)")
                        nc.vector.tensor_mul(eqf, eqf, wqt[:])
                        r = wp.tile([P, AT], bf16, tag="r")
                        nc.vector.reduce_max(out=r[:], in_=eq[:], axis=X)

                        ci = wp.tile([P, AT], i32, tag="ci")
                        nc.scalar.activation(
                            out=ci[:], in_=r[:], func=Act.Identity, bias=b21[:],
                            scale=-1.0,
                        )
                        stq.dma_start(out=cv[:, t, :], in_=ci[:])
                        # valid; per-half-row counts (batch = f(partition, half))
                        vld = wp.tile([P, AT], u8, tag="vld")
                        for s in range(2):
                            nc.scalar.activation(
                                out=vld[:, s * HALF : (s + 1) * HALF],
                                in_=r[:, s * HALF : (s + 1) * HALF],
                                func=Act.Sign, bias=b21[:], scale=-1.0,
                                accum_out=numpart[:, s, t : t + 1],
                            )
                        stq.dma_start(out=vv[:, t, :], in_=vld[:])
                    else:  # keep identical DMA traffic
                        stq.dma_start(out=cv[:, t, :], in_=di32[:])
                        stq.dma_start(out=vv[:, t, :], in_=du8[:])
                    if "scores" not in blocks:
                        stq.dma_start(out=sv[:, t, :], in_=df32[:, 0:AT])

                    # ---- box decode ----
                    if "decode" in blocks:
                        xy = wp.tile([P, AT, 2], f32, tag="xy")
                        nc.gpsimd.tensor_mul(xy[:], loc01, w1t[:])
                        nc.gpsimd.tensor_add(xy[:], xy[:], c1t[:])
                        bw = wp.tile([P, AT, 2], f32, tag="bw")
                        nc.scalar.activation(
                            out=bw[:].rearrange("p a c -> p (a c)"),
                            in_=loc23, func=Act.Exp,
                        )
                        nc.gpsimd.tensor_mul(bw[:], bw[:], l1t[:])
                        bx = wp.tile([P, AT, 4], f32, tag="bx")
                        nc.gpsimd.tensor_sub(bx[:, :, 0:2], xy[:], bw[:])
                        nc.gpsimd.tensor_add(bx[:, :, 2:4], xy[:], bw[:])
                        bxf = bx[:].rearrange("p a c -> p (a c)")
                        nc.vector.tensor_scalar(
                            out=bxf, in0=bxf, scalar1=0.0, scalar2=1.0,
                            op0=Alu.max, op1=Alu.min,
                        )
                        stq.dma_start(out=bv[:, t, :], in_=bxf)
                    else:
                        stq.dma_start(out=bv[:, t, :], in_=df32[:])

            # ---- num[bb, t] = sum_p sum_s msk[p, s*4+bb] * numpart[p, s, t]
            if "num" in blocks and "argmax" in blocks:
                ps = pp.tile([BPT, NTILES], f32, tag="ps")
                nc.tensor.matmul(
                    ps[:], mkt[:, 0:BPT], numpart[:, 0, :],
                    start=True, stop=False,
                )
                nc.tensor.matmul(
                    ps[:], mkt[:, BPT : 2 * BPT], numpart[:, 1, :],
                    start=False, stop=True,
                )
                numi = consts.tile([BPT, NTILES], i32, tag="numi")
                nc.scalar.copy(out=numi[:], in_=ps[:])
                stq.dma_start(out=nvt, in_=numi[:])
            else:
                numi = consts.tile([BPT, NTILES], i32, tag="numi")
                nc.vector.memset(numi[:], 0)
                stq.dma_start(out=nvt, in_=numi[:])

    nc.compile()
    return nc


def host_tables(anchors):
    a = np.asarray(anchors, np.float32)
    cxcy = (a[:, 2:4] + a[:, 0:2]) * 0.5
    wh = a[:, 2:4] - a[:, 0:2]
    wh2 = wh * 0.5  # bwh/2 = exp(loc23) * wh/2

    # anchor id for (partition, slot): tile-invariant since P*AT % N == 0
    idx = (np.arange(P)[:, None] * AT + np.arange(AT)[None, :]) % N  # [P, AT]

    def lay(t):  # [N,2] -> [P, AT*2]
        return np.ascontiguousarray(t[idx]).reshape(P, AT * 2)

    wvals = np.array([NCLS - j for j in range(NCLS)] + [0], np.float32)
    wq = np.tile(wvals[None, None], (P, AT, 1)).reshape(P, AT * CP)

    msk = np.zeros((P, 2 * BPT), np.float32)
    for p in range(P):
        for s in range(2):
            msk[p, s * BPT + batch_of_half(p, s)] = 1.0

    return {
        "w1": lay(wh),
        "c1": lay(cxcy),
        "l1": lay(wh2),
        "wq": wq.astype(ml_dtypes.bfloat16),
        "msk": msk,
    }


LAST_RESULT = None


def kernel(logits, anchors):
    global LAST_RESULT
    import os
    from concourse.bass_utils import run_bass_kernel_spmd
    from concourse._compat import axon_active

    if axon_active():
        try:
            from antenv.axon_hooks import get_axon_ntff_profile_hook  # noqa: F401
        except ImportError:
            # BASS_TRACE under axon would crash on this missing module
            os.environ["BASS_NEVER_TRACE"] = "1"

    if "nc" not in _CACHE:
        _CACHE["nc"] = build_program()
    nc = _CACHE["nc"]

    logits = np.ascontiguousarray(np.asarray(logits, np.float32))
    tables = host_tables(anchors)
    shards = logits.reshape(NCORES, BPC, N, NCH)
    in_maps = [
        {"logits": np.ascontiguousarray(shards[i]), **tables}
        for i in range(NCORES)
    ]
    res = run_bass_kernel_spmd(nc, in_maps, core_ids=list(range(NCORES)))
    LAST_RESULT = res

    boxes = np.concatenate([r["boxes"] for r in res.results], axis=0)
    scores = np.concatenate([r["scores"] for r in res.results], axis=0)
    classes = np.concatenate([r["classes"] for r in res.results], axis=0)
    valid = np.concatenate([r["valid"] for r in res.results], axis=0)
    num = np.concatenate([r["num"] for r in res.results], axis=0)
    return boxes, scores, classes.astype(np.int32), valid.astype(bool), num.astype(np.int32)
